# revision 1
# baseline (speedup 1.0000x reference)
# CARAFE (content-aware reassembly) Trainium2 Bass kernel.
# Strategy: data-parallel over batch (8 items -> 8 NeuronCores). Per core:
#   - 1x1 compressor conv (PE matmul, fp32) + folded BN + SiLU (ACT)
#   - 3x3 encoder conv as 9 accumulating matmuls on padded t (PE, fp32)
#     + folded BN; exp + per-class normalization for the 25-way softmax
#   - reassembly split by tap row parity:
#       * 15 even-row taps run on PE as mask-diagonal matmuls accumulated
#         into the output-transpose PSUM group. The diagonal matrices for
#         all (j, i, cl) of a tile are built by ONE batched gpsimd
#         affine_select per cl from column-shifted mask copies (fp16).
#       * 10 odd-row taps run on DVE as per-pixel fused multiply-accumulate
#         (scalar_tensor_tensor) against partition-shifted xT windows
#         (fp16) produced by shift-matrix matmuls on PE.
#   - output is produced channel-major by the PE accumulation, staged
#     (ACT) to interleave the subpixel classes, and DMA'd out in fp32.
import sys
import numpy as np

for _p in ("/opt/trn_rl_repo",):
    if _p not in sys.path:
        sys.path.insert(0, _p)

B, C, Cm, E = 8, 192, 64, 100
H = W = 64
K, S = 5, 2
EPS = 1e-3
NT = 32            # pixel tiles (2 rows x 64 cols = 128 pixels each)
NB = 36            # xT row blocks of 128 (rows r in [-4, 68))

# shift-matrix offsets tau: S_tau[k, m] = 1 iff k == m + tau
TAUS = sorted({0, 1, 2, 62, 63, 64, 65, 66, 126, 127,
               -1, -2, -62, -63, -64, -65, -66, -126, -127})
TAU_IDX = {t: i for i, t in enumerate(TAUS)}

EVEN_TAPS = [i * K + j for i in (0, 2, 4) for j in range(K)]   # PE taps
ODD_TAPS = [i * K + j for i in (1, 3) for j in range(K)]       # DVE taps

# colsB layout: shifted mask-column set for shift j lives at flat offset
# 132*j (so channel ch of shift j sits at 136*j + 40*(i//2) + cl, affine in
# (j, i, cl) -- see the batched affine_select below).
COLS_ROW = 132
COLS_JSTRIDE = 136
COLS_FLAT = 640

_prog_cache = {}


def _build_program(num_devices=8):
    import concourse.mybir as mybir
    import concourse.tile as tile
    from concourse import bacc
    from contextlib import ExitStack

    fp32 = mybir.dt.float32
    fp16 = mybir.dt.float16
    AL = mybir.AluOpType
    AF = mybir.ActivationFunctionType

    nc = bacc.Bacc("TRN2", target_bir_lowering=False, num_devices=num_devices)

    x_d = nc.dram_tensor("x", [C, H * W], fp32, kind="ExternalInput").ap()
    cw_d = nc.dram_tensor("cw", [C, Cm], fp32, kind="ExternalInput").ap()
    cb_d = nc.dram_tensor("cb", [Cm, 1], fp32, kind="ExternalInput").ap()
    ewp_d = nc.dram_tensor("ewp", [128, 3 * E], fp32, kind="ExternalInput").ap()
    ew2_d = nc.dram_tensor("ew2", [Cm, 3 * E], fp32, kind="ExternalInput").ap()
    eb_d = nc.dram_tensor("eb", [E, 1], fp32, kind="ExternalInput").ap()
    edge_d = nc.dram_tensor("edge", [128, K * K], fp32, kind="ExternalInput").ap()
    shm_d = nc.dram_tensor("shm", [128, len(TAUS) * 128], fp16, kind="ExternalInput").ap()
    id32_d = nc.dram_tensor("id32", [128, 128], fp32, kind="ExternalInput").ap()
    out_d = nc.dram_tensor("out", [C, H, S, S * W], fp32, kind="ExternalOutput").ap()

    es = ExitStack()
    with tile.TileContext(nc) as tc:
        with es:
            _body(es, tc, nc, mybir, fp32, fp16, AL, AF,
                  x_d, cw_d, cb_d, ewp_d, ew2_d, eb_d, edge_d, shm_d, id32_d, out_d)
    nc.compile()
    return nc


def _body(es, tc, nc, mybir, fp32, fp16, AL, AF,
          x_d, cw_d, cb_d, ewp_d, ew2_d, eb_d, edge_d, shm_d, id32_d, out_d):
    import bass_rust

    def raw_ap(ap, dims, extra_offset=0):
        """View `ap` (a full-tile [128, F] AP) with explicit free dims."""
        c = ap.copy()
        c.ap = bass_rust.VecI64Pair([list(ap.ap[0])] + [list(d) for d in dims])
        c.offset = ap.offset + extra_offset
        return c

    consts = es.enter_context(tc.tile_pool(name="consts", bufs=1))
    big = es.enter_context(tc.tile_pool(name="big", bufs=1))

    cw0 = consts.tile([128, Cm], fp32, tag="cw0")
    cw1 = consts.tile([64, Cm], fp32, tag="cw1")
    cb = consts.tile([Cm, 1], fp32, tag="cb")
    ewp = consts.tile([128, 3 * E], fp32, tag="ewp")
    ew2 = consts.tile([Cm, 3 * E], fp32, tag="ew2")
    eb = consts.tile([E, 1], fp32, tag="eb")
    edge = consts.tile([128, K * K], fp32, tag="edge")
    shm = consts.tile([128, len(TAUS), 128], fp16, tag="shm")
    id32 = consts.tile([128, 128], fp32, tag="id32")

    fp32r = mybir.dt.float32r
    x0 = big.tile([128, H * W], fp32, tag="x0")
    x1 = big.tile([64, H * W], fp32, tag="x1")
    t_pad = big.tile([128, 66 * 66], fp32, tag="tpad")
    e_sb = big.tile([E, H * W], fp32, tag="esb")
    xT = big.tile([128, NB, C], fp16, tag="xT")
    xT2 = big.tile([128, NB - 1, C], fp16, tag="xT2")
    maskT = big.tile([128, NT, E], fp32, tag="maskT")
    rsum = big.tile([128, NT, 4], fp32, tag="rsum")

    nc.sync.dma_start(out=cw0[:], in_=cw_d[0:128, :])
    nc.sync.dma_start(out=cw1[:], in_=cw_d[128:192, :])
    nc.sync.dma_start(out=cb[:], in_=cb_d)
    nc.sync.dma_start(out=ewp[:], in_=ewp_d)
    nc.sync.dma_start(out=ew2[:], in_=ew2_d)
    nc.sync.dma_start(out=eb[:], in_=eb_d)
    nc.sync.dma_start(out=edge[:], in_=edge_d)
    nc.sync.dma_start(out=shm[:].rearrange("p a b -> p (a b)"), in_=shm_d)
    nc.sync.dma_start(out=id32[:], in_=id32_d)
    for ck in range(8):
        c0 = ck * 512
        nc.sync.dma_start(out=x0[:, c0:c0 + 512], in_=x_d[0:128, c0:c0 + 512])
        nc.sync.dma_start(out=x1[:, c0:c0 + 512], in_=x_d[128:192, c0:c0 + 512])

    ident16 = shm[:, TAU_IDX[0], :]  # [128, 128] fp16 identity

    # zero borders of t_pad and the vertical zero blocks of xT
    nc.gpsimd.memset(t_pad[:], 0.0)
    nc.gpsimd.memset(xT[:, 0:2, :], 0.0)
    nc.gpsimd.memset(xT[:, NB - 2:NB, :], 0.0)

    # All PSUM pools stay open concurrently (8 banks total) so the stack
    # allocator never reuses addresses across phases (false deps would
    # serialize the phases).
    win_ps = es.enter_context(tc.tile_pool(name="win_ps", bufs=2, space="PSUM"))
    conv_ps = es.enter_context(tc.tile_pool(name="conv_ps", bufs=1, space="PSUM"))
    mt_ps = es.enter_context(tc.tile_pool(name="mt_ps", bufs=2, space="PSUM"))
    out_ps = es.enter_context(tc.tile_pool(name="out_ps", bufs=3, space="PSUM"))
    c1sg = es.enter_context(tc.tile_pool(name="c1sg", bufs=2))
    work = es.enter_context(tc.tile_pool(name="work", bufs=30))
    accp = es.enter_context(tc.tile_pool(name="accp", bufs=12))
    stagep = es.enter_context(tc.tile_pool(name="stagep", bufs=6))
    colsp = es.enter_context(tc.tile_pool(name="colsp", bufs=3))
    swsp = es.enter_context(tc.tile_pool(name="swsp", bufs=6))

    # ---- transpose x into xT (row blocks offset by +256 rows of zero pad),
    # cast to fp16 on PSUM eviction ----
    for pb in range(32):  # pixel blocks of 128
        p0 = pb * 128
        q = pb + 2
        pt0 = win_ps.tile([128, C], fp32, name="pt0", tag="winps")
        nc.tensor.transpose(pt0[:, 0:128], x0[:, p0:p0 + 128], id32[:])
        nc.scalar.copy(out=xT[:, q, 0:128], in_=pt0[:, 0:128])
        pt1 = win_ps.tile([128, C], fp32, name="pt1", tag="winps")
        nc.tensor.transpose(pt1[:, 0:64], x1[:, p0:p0 + 128], id32[0:64, 0:64])
        nc.scalar.copy(out=xT[:, q, 128:192], in_=pt1[:, 0:64])
        # xT2 block b = padded rows (2b+1, 2b+2): lower half from xT[64:, b],
        # upper half from xT[0:64, b+1] (partition-shift DMAs, chunked so
        # early blocks are available early).
        if pb in (7, 15, 23, 31):
            lo = {7: 0, 15: 9, 23: 17, 31: 25}[pb]
            hi = {7: 9, 15: 17, 23: 25, 31: 35}[pb]
            nc.sync.dma_start(out=xT2[0:64, lo:hi, :], in_=xT[64:128, lo:hi, :])
            nc.sync.dma_start(out=xT2[64:128, lo:hi, :], in_=xT[0:64, lo + 1:hi + 1, :])

    def conv1(nt):
        n0 = nt * 512
        ps = conv_ps.tile([E, 512], fp32, name="c1ps", tag="conv")
        nc.tensor.matmul(ps[0:Cm, :], cw0[:], x0[:, n0:n0 + 512], start=True, stop=False)
        nc.tensor.matmul(ps[0:Cm, :], cw1[:], x1[:, n0:n0 + 512], start=False, stop=True)
        # silu(y) = y*sigmoid(y) with y = ps + cb
        sg = c1sg.tile([Cm, 512], fp32, tag="sg")
        nc.scalar.activation(out=sg[:], in_=ps[0:Cm, :], func=AF.Sigmoid, bias=cb[:], scale=1.0)
        v = t_pad[0:Cm, :].rearrange("c (r z) -> c r z", z=66)[:, nt * 8 + 1: nt * 8 + 9, 1:65]

        nc.vector.scalar_tensor_tensor(
            v, ps[0:Cm, :].rearrange("c (r z) -> c r z", z=64), cb[:],
            sg[:].rearrange("c (r z) -> c r z", z=64), AL.add, AL.mult)
        # upper half holds t shifted up one row (for the dy0/dy1 tap pairing)
        nc.sync.dma_start(
            out=t_pad[64:128, (nt * 8) * 66:(nt * 8 + 8) * 66],
            in_=t_pad[0:Cm, (nt * 8 + 1) * 66:(nt * 8 + 9) * 66])

    def conv2(nt):
        r0 = nt * 8
        ps = conv_ps.tile([E, 512], fp32, name="c2ps", tag="conv")
        for dx in range(3):
            rhsp = t_pad[:].rearrange("c (r z) -> c r z", z=66)[:, r0: r0 + 8, dx: dx + 64]
            nc.tensor.matmul(ps[:], ewp[:, dx * E:(dx + 1) * E], rhsp,
                             start=(dx == 0), stop=False)
            rhs2 = t_pad[0:Cm, :].rearrange("c (r z) -> c r z", z=66)[:, r0 + 2: r0 + 10, dx: dx + 64]
            nc.tensor.matmul(ps[:], ew2[:, dx * E:(dx + 1) * E], rhs2,
                             start=False, stop=(dx == 2))
        nc.scalar.activation(out=e_sb[:, nt * 512:(nt + 1) * 512], in_=ps[:],
                             func=AF.Exp, bias=eb[:], scale=1.0)

    def mask_tile(ti):
        p0 = ti * 128
        pt = mt_ps.tile([128, E], fp32, name="mt", tag="mt")
        nc.tensor.transpose(pt[:], e_sb[:, p0:p0 + 128], id32[0:E, 0:E])
        nc.scalar.copy(out=maskT[:, ti, :], in_=pt[:])
        # maskT free layout: ch = ij*4 + cl
        v_cl_ij = maskT[:, ti, :].rearrange("p (ij cl) -> p cl ij", cl=4)
        s = rsum[:, ti, :]
        nc.vector.tensor_reduce(out=s, in_=v_cl_ij, axis=mybir.AxisListType.X, op=AL.add)
        nc.vector.reciprocal(s, s)
        e_cl_ij = edge[:].unsqueeze(1).broadcast_to([128, 4, K * K])
        nc.vector.tensor_tensor(v_cl_ij, v_cl_ij, e_cl_ij, AL.mult)
        v_ij_cl = maskT[:, ti, :].rearrange("p (ij cl) -> p ij cl", cl=4)
        r_b = rsum[:, ti, :].unsqueeze(1).broadcast_to([128, K * K, 4])
        nc.vector.tensor_tensor(v_ij_cl, v_ij_cl, r_b, AL.mult)

    # ---- main reassembly ----

    win_cache = {}

    def get_window(r, dj):
        # odd padded row r: window rows (r, r+1) live in xT2 block (r-1)//2
        key = (r, dj)
        if key not in win_cache:
            b = (r - 1) // 2
            if dj == 0:
                win_cache[key] = xT2[:, b, :]
            else:
                ps = win_ps.tile([128, C], fp32, tag="winps")
                nc.tensor.matmul(ps[:], shm[:, TAU_IDX[dj], :], xT2[:, b, :],
                                 start=True, stop=True)
                w_sb = work.tile([128, C], fp16, tag="win")
                nc.scalar.copy(out=w_sb[:], in_=ps[:])
                win_cache[key] = w_sb[:]
        return win_cache[key]

    preps = {}

    def tile_prep(ti):
        nonlocal win_cache
        h0 = ti * 2
        # windows for the odd-row DVE taps (i = 1, 3)
        wins = {}
        for i in (1, 3):
            for j in range(K):
                wins[i * K + j] = get_window(h0 + i + 2, j - 2)
        # retire windows no longer needed (keep pool pressure bounded)
        win_cache = {k: v for k, v in win_cache.items() if k[0] >= h0 + 3}

        # column-shifted mask copies for the even-row PE taps:
        # colsB[.., 132*j : 132*j+100] = maskT[:, ti] shifted by -(j-2)
        # partitions (cast to fp16).
        colsB = colsp.tile([128, COLS_FLAT], fp16, name="colsB", tag="colsB")
        nc.scalar.copy(out=colsB[:, 2 * COLS_ROW:2 * COLS_ROW + E],
                       in_=maskT[:, ti, :])
        for j in (0, 1, 3, 4):
            t = 2 - j
            cps = mt_ps.tile([128, E], fp32, name="cps", tag="mt")
            nc.tensor.matmul(cps[:], shm[:, TAU_IDX[t], :],
                             colsB[:, 2 * COLS_ROW:2 * COLS_ROW + E],
                             start=True, stop=True)
            nc.scalar.copy(out=colsB[:, j * COLS_ROW:j * COLS_ROW + E], in_=cps[:])

        # diagonal matrices sws[q, j, a, p] = cols_j[q] if q == p + (j-2):
        # j != 2 via two batched gpsimd affine_selects per cl; j == 2 (no
        # column shift) on ACT as identity-scaled copies (scale = mask col).
        sws_cl = []
        for cl in range(4):
            sws = swsp.tile([128, K, 3, 128], fp16, name=f"sws{cl}", tag="sws")
            for j in (0, 1, 3, 4):
                in_ap = raw_ap(colsB[:], [[40, 3], [0, 128]],
                               extra_offset=cl + j * COLS_JSTRIDE)
                nc.gpsimd.affine_select(
                    out=sws[:, j, :, :], in_=in_ap,
                    compare_op=AL.is_equal, fill=0.0, base=2 - j,
                    channel_multiplier=1, pattern=[[0, 3], [-1, 128]])
            for a in range(3):
                ch = (10 * a + 2) * 4 + cl
                nc.scalar.activation(out=sws[:, 2, a, :], in_=ident16,
                                     func=AF.Copy, bias=0.0,
                                     scale=maskT[:, ti, ch:ch + 1])
            sws_cl.append(sws)
        preps[ti] = (wins, sws_cl)

    def tile_out(ti):
        h0 = ti * 2
        wins, sws_cl = preps.pop(ti)
        stg_tiles = {(di, ch): stagep.tile([96, 2, S * W], fp32, name=f"stg{di}_{ch}", tag=f"stg{di}_{ch}")
                     for di in range(2) for ch in range(2)}
        for di in range(2):
            accs = []
            for dj in range(2):
                cl = di * 2 + dj
                # odd-row taps: DVE fused multiply-accumulate in fp16
                acc = accp.tile([128, C], fp16, tag="acc")
                first = True
                for ij in ODD_TAPS:
                    col = maskT[:, ti, ij * 4 + cl:ij * 4 + cl + 1]
                    if first:
                        nc.vector.tensor_scalar(acc[:], wins[ij], col, None, AL.mult)
                        first = False
                    else:
                        nc.vector.scalar_tensor_tensor(acc[:], wins[ij], col, acc[:],
                                                       AL.mult, AL.add)
                accs.append((dj, cl, acc))
            # transpose accs -> [c, pix] and accumulate the even-row taps on
            # PE, writing the two dj subpixel columns interleaved (stride 2)
            # into one PSUM tile so a single ACT copy stages both.
            for ch in range(2):
                c0 = ch * 96
                ptp = out_ps.tile([96, 2 * 128], fp32, tag="ot")
                for dj, cl, acc in accs:
                    ptv = ptp[:].rearrange("c (p t) -> c p t", t=2)[:, :, dj]
                    sws = sws_cl[cl]
                    nc.tensor.matmul(ptv, acc[:, c0:c0 + 96], ident16,
                                     start=True, stop=False,
                                     skip_group_check=True)
                    for i in (0, 2, 4):
                        q = (h0 + i + 2) // 2
                        for j in range(K):
                            nc.tensor.matmul(ptv, xT[:, q, c0:c0 + 96],
                                             sws[:, j, i // 2, :],
                                             start=False,
                                             stop=(i == 4 and j == K - 1),
                                             skip_group_check=True)
                stg = stg_tiles[(di, ch)]
                nc.scalar.copy(out=stg[:], in_=ptp[:].rearrange("c (h w) -> c h w", h=2))
                nc.sync.dma_start(
                    out=out_d[c0:c0 + 96, h0:h0 + 2, di, :],
                    in_=stg[:])

    # interleave convs and mask tiles; emit the first two tiles' prep
    # (windows + colsB + diagonal builds) early so the Pool/ACT reassembly
    # pipeline starts during the conv phase instead of after it.
    conv1(0)
    for nt in range(8):
        if nt + 1 < 8:
            conv1(nt + 1)
        conv2(nt)
        for sub in range(4):
            ti = nt * 4 + sub
            mask_tile(ti)
            if ti < 2:
                tile_prep(ti)
    for ti in range(NT):
        if ti not in preps:
            tile_prep(ti)
        tile_out(ti)
    es.pop_all().close()


def _host_prep(inputs):
    def fold(w, g, b, m, v):
        s = g / np.sqrt(v + EPS)
        return (w * s[:, None, None, None]).astype(np.float32), (b - m * s).astype(np.float32)

    comp_w_eff, comp_b_eff = fold(inputs["comp_w"], inputs["comp_g"], inputs["comp_b"],
                                  inputs["comp_m"], inputs["comp_v"])
    enc_w_eff, enc_b_eff = fold(inputs["enc_w"], inputs["enc_g"], inputs["enc_b"],
                                inputs["enc_m"], inputs["enc_v"])
    cw = np.ascontiguousarray(comp_w_eff[:, :, 0, 0].T)          # [192, 64]
    cb = comp_b_eff.reshape(Cm, 1)
    ewp = np.concatenate([np.concatenate([enc_w_eff[:, :, 0, dx].T,
                                          enc_w_eff[:, :, 1, dx].T], axis=0)
                          for dx in range(3)], axis=1)  # [128, 300]
    ewp = np.ascontiguousarray(ewp)
    ew2 = np.ascontiguousarray(np.concatenate(
        [enc_w_eff[:, :, 2, dx].T for dx in range(3)], axis=1))  # [64, 300]
    eb = enc_b_eff.reshape(E, 1)
    wv = np.arange(128) % 64
    edge = np.zeros((128, K * K), np.float32)
    for j in range(K):
        ok = (wv + j - 2 >= 0) & (wv + j - 2 < W)
        for i in range(K):
            edge[:, i * K + j] = ok
    shm = np.zeros((128, len(TAUS), 128), np.float16)
    for t, i in TAU_IDX.items():
        shm[:, i, :] = np.eye(128, dtype=np.float16, k=-t)
    shm = shm.reshape(128, len(TAUS) * 128)
    id32 = np.eye(128, dtype=np.float32)
    return dict(cw=cw, cb=cb, ewp=ewp, ew2=ew2, eb=eb, edge=edge, shm=shm, id32=id32)


def kernel(**inputs):
    from concourse.bass_utils import run_bass_kernel_spmd

    inputs = {k: np.asarray(v, dtype=np.float32) for k, v in inputs.items()}
    w = _host_prep(inputs)
    if "nc" not in _prog_cache:
        _prog_cache["nc"] = _build_program()
    nc = _prog_cache["nc"]
    x = inputs["x"]
    in_maps = [dict(x=np.ascontiguousarray(x[b].reshape(C, H * W)), **w) for b in range(B)]
    res = run_bass_kernel_spmd(nc, in_maps, list(range(B)))
    out = np.stack([res.results[b]["out"].reshape(C, 2 * H, 2 * W) for b in range(B)])
    return out



# revision 4
# speedup vs baseline: 1.0187x; 1.0187x over previous
# CARAFE (content-aware reassembly) Trainium2 Bass kernel.
# Strategy: data-parallel over batch (8 items -> 8 NeuronCores). Per core:
#   - 1x1 compressor conv (PE matmul, fp32) + folded BN + SiLU (ACT)
#   - 3x3 encoder conv as 9 accumulating matmuls on padded t (PE, fp32)
#     + folded BN; exp + per-class normalization for the 25-way softmax
#   - reassembly split by tap row parity:
#       * 15 even-row taps run on PE as mask-diagonal matmuls accumulated
#         into the output-transpose PSUM group. The diagonal matrices for
#         all (j, i, cl) of a tile are built by ONE batched gpsimd
#         affine_select per cl from column-shifted mask copies (fp16).
#       * 10 odd-row taps run on DVE as per-pixel fused multiply-accumulate
#         (scalar_tensor_tensor) against partition-shifted xT windows
#         (fp16) produced by shift-matrix matmuls on PE.
#   - output is produced channel-major by the PE accumulation, staged
#     (ACT) to interleave the subpixel classes, and DMA'd out in fp32.
import sys
import numpy as np

for _p in ("/opt/trn_rl_repo",):
    if _p not in sys.path:
        sys.path.insert(0, _p)

B, C, Cm, E = 8, 192, 64, 100
H = W = 64
K, S = 5, 2
EPS = 1e-3
NT = 32            # pixel tiles (2 rows x 64 cols = 128 pixels each)
NB = 36            # xT row blocks of 128 (rows r in [-4, 68))

# shift-matrix offsets tau: S_tau[k, m] = 1 iff k == m + tau
TAUS = sorted({0, 1, 2, 62, 63, 64, 65, 66, 126, 127,
               -1, -2, -62, -63, -64, -65, -66, -126, -127})
TAU_IDX = {t: i for i, t in enumerate(TAUS)}

EVEN_TAPS = [i * K + j for i in (0, 2, 4) for j in range(K)]   # PE taps
ODD_TAPS = [i * K + j for i in (1, 3) for j in range(K)]       # DVE taps

# colsB layout: shifted mask-column set for shift j lives at flat offset
# 132*j (so channel ch of shift j sits at 136*j + 40*(i//2) + cl, affine in
# (j, i, cl) -- see the batched affine_select below).
COLS_ROW = 132
COLS_JSTRIDE = 136
COLS_FLAT = 640

_prog_cache = {}


def _build_program(num_devices=8):
    import concourse.mybir as mybir
    import concourse.tile as tile
    from concourse import bacc
    from contextlib import ExitStack

    fp32 = mybir.dt.float32
    fp32r = mybir.dt.float32r
    fp16 = mybir.dt.float16
    AL = mybir.AluOpType
    AF = mybir.ActivationFunctionType

    nc = bacc.Bacc("TRN2", target_bir_lowering=False, num_devices=num_devices)

    x_d = nc.dram_tensor("x", [C, H * W], fp32r, kind="ExternalInput").ap()
    cw_d = nc.dram_tensor("cw", [C, Cm], fp32r, kind="ExternalInput").ap()
    cb_d = nc.dram_tensor("cb", [Cm, 1], fp32, kind="ExternalInput").ap()
    ewp_d = nc.dram_tensor("ewp", [128, 3 * E], fp32r, kind="ExternalInput").ap()
    ew2_d = nc.dram_tensor("ew2", [Cm, 3 * E], fp32r, kind="ExternalInput").ap()
    eb_d = nc.dram_tensor("eb", [E, 1], fp32, kind="ExternalInput").ap()
    edge_d = nc.dram_tensor("edge", [128, K * K], fp32, kind="ExternalInput").ap()
    shm_d = nc.dram_tensor("shm", [128, len(TAUS) * 128], fp16, kind="ExternalInput").ap()
    id32_d = nc.dram_tensor("id32", [128, 128], fp32r, kind="ExternalInput").ap()
    out_d = nc.dram_tensor("out", [C, H, S, S * W], fp32, kind="ExternalOutput").ap()

    es = ExitStack()
    with tile.TileContext(nc) as tc:
        with es:
            _body(es, tc, nc, mybir, fp32, fp16, AL, AF,
                  x_d, cw_d, cb_d, ewp_d, ew2_d, eb_d, edge_d, shm_d, id32_d, out_d)
    nc.compile()
    return nc


def _body(es, tc, nc, mybir, fp32, fp16, AL, AF,
          x_d, cw_d, cb_d, ewp_d, ew2_d, eb_d, edge_d, shm_d, id32_d, out_d):
    import bass_rust
    fp32r = mybir.dt.float32r

    def raw_ap(ap, dims, extra_offset=0):
        """View `ap` (a full-tile [128, F] AP) with explicit free dims."""
        c = ap.copy()
        c.ap = bass_rust.VecI64Pair([list(ap.ap[0])] + [list(d) for d in dims])
        c.offset = ap.offset + extra_offset
        return c

    consts = es.enter_context(tc.tile_pool(name="consts", bufs=1))
    big = es.enter_context(tc.tile_pool(name="big", bufs=1))

    cw0 = consts.tile([128, Cm], fp32r, tag="cw0")
    cw1 = consts.tile([64, Cm], fp32r, tag="cw1")
    cb = consts.tile([Cm, 1], fp32, tag="cb")
    ewp = consts.tile([128, 3 * E], fp32r, tag="ewp")
    ew2 = consts.tile([Cm, 3 * E], fp32r, tag="ew2")
    eb = consts.tile([E, 1], fp32, tag="eb")
    edge = consts.tile([128, K * K], fp32, tag="edge")
    shm = consts.tile([128, len(TAUS), 128], fp16, tag="shm")
    id32 = consts.tile([128, 128], fp32r, tag="id32")

    fp32r = mybir.dt.float32r
    x0 = big.tile([128, H * W], fp32r, tag="x0")
    x1 = big.tile([64, H * W], fp32r, tag="x1")
    t_pad = big.tile([128, 66 * 66], fp32r, tag="tpad")
    e_sb = big.tile([E, H * W], fp32r, tag="esb")
    xT = big.tile([128, NB, C], fp16, tag="xT")
    xT2 = big.tile([128, NB - 1, C], fp16, tag="xT2")
    maskT = big.tile([128, NT, E], fp32, tag="maskT")
    rsum = big.tile([128, NT, 4], fp32, tag="rsum")

    nc.sync.dma_start(out=cw0[:], in_=cw_d[0:128, :])
    nc.sync.dma_start(out=cw1[:], in_=cw_d[128:192, :])
    nc.sync.dma_start(out=cb[:], in_=cb_d)
    nc.sync.dma_start(out=ewp[:], in_=ewp_d)
    nc.sync.dma_start(out=ew2[:], in_=ew2_d)
    nc.sync.dma_start(out=eb[:], in_=eb_d)
    nc.sync.dma_start(out=edge[:], in_=edge_d)
    nc.sync.dma_start(out=shm[:].rearrange("p a b -> p (a b)"), in_=shm_d)
    nc.sync.dma_start(out=id32[:], in_=id32_d)
    for ck in range(8):
        c0 = ck * 512
        nc.sync.dma_start(out=x0[:, c0:c0 + 512], in_=x_d[0:128, c0:c0 + 512])
        nc.sync.dma_start(out=x1[:, c0:c0 + 512], in_=x_d[128:192, c0:c0 + 512])

    ident16 = shm[:, TAU_IDX[0], :]  # [128, 128] fp16 identity

    # zero borders of t_pad and the vertical zero blocks of xT
    nc.gpsimd.memset(t_pad[:], 0.0)
    nc.gpsimd.memset(xT[:, 0:2, :], 0.0)
    nc.gpsimd.memset(xT[:, NB - 2:NB, :], 0.0)

    # All PSUM pools stay open concurrently (8 banks total) so the stack
    # allocator never reuses addresses across phases (false deps would
    # serialize the phases).
    win_ps = es.enter_context(tc.tile_pool(name="win_ps", bufs=2, space="PSUM"))
    conv_ps = es.enter_context(tc.tile_pool(name="conv_ps", bufs=1, space="PSUM"))
    mt_ps = es.enter_context(tc.tile_pool(name="mt_ps", bufs=2, space="PSUM"))
    out_ps = es.enter_context(tc.tile_pool(name="out_ps", bufs=3, space="PSUM"))
    c1sg = es.enter_context(tc.tile_pool(name="c1sg", bufs=2))
    work = es.enter_context(tc.tile_pool(name="work", bufs=30))
    accp = es.enter_context(tc.tile_pool(name="accp", bufs=12))
    stagep = es.enter_context(tc.tile_pool(name="stagep", bufs=6))
    colsp = es.enter_context(tc.tile_pool(name="colsp", bufs=3))
    swsp = es.enter_context(tc.tile_pool(name="swsp", bufs=6))

    # ---- transpose x into xT (row blocks offset by +256 rows of zero pad),
    # cast to fp16 on PSUM eviction ----
    for pb in range(32):  # pixel blocks of 128
        p0 = pb * 128
        q = pb + 2
        pt0 = win_ps.tile([128, C], fp32r, name="pt0", tag="winps")
        nc.tensor.transpose(pt0[:, 0:128], x0[:, p0:p0 + 128], id32[:])
        nc.scalar.copy(out=xT[:, q, 0:128], in_=pt0[:, 0:128])
        pt1 = win_ps.tile([128, C], fp32r, name="pt1", tag="winps")
        nc.tensor.transpose(pt1[:, 0:64], x1[:, p0:p0 + 128], id32[0:64, 0:64])
        nc.scalar.copy(out=xT[:, q, 128:192], in_=pt1[:, 0:64])
        # xT2 block b = padded rows (2b+1, 2b+2): lower half from xT[64:, b],
        # upper half from xT[0:64, b+1] (partition-shift DMAs, chunked so
        # early blocks are available early).
        if pb in (7, 15, 23, 31):
            lo = {7: 0, 15: 9, 23: 17, 31: 25}[pb]
            hi = {7: 9, 15: 17, 23: 25, 31: 35}[pb]
            nc.sync.dma_start(out=xT2[0:64, lo:hi, :], in_=xT[64:128, lo:hi, :])
            nc.sync.dma_start(out=xT2[64:128, lo:hi, :], in_=xT[0:64, lo + 1:hi + 1, :])

    def conv1(nt):
        n0 = nt * 512
        ps = conv_ps.tile([E, 512], fp32, name="c1ps", tag="conv")
        nc.tensor.matmul(ps[0:Cm, :], cw0[:], x0[:, n0:n0 + 512], start=True, stop=False)
        nc.tensor.matmul(ps[0:Cm, :], cw1[:], x1[:, n0:n0 + 512], start=False, stop=True)
        # silu(y) = y*sigmoid(y) with y = ps + cb
        sg = c1sg.tile([Cm, 512], fp32, tag="sg")
        nc.scalar.activation(out=sg[:], in_=ps[0:Cm, :], func=AF.Sigmoid, bias=cb[:], scale=1.0)
        v = t_pad[0:Cm, :].rearrange("c (r z) -> c r z", z=66)[:, nt * 8 + 1: nt * 8 + 9, 1:65]

        nc.vector.scalar_tensor_tensor(
            v, ps[0:Cm, :].rearrange("c (r z) -> c r z", z=64), cb[:],
            sg[:].rearrange("c (r z) -> c r z", z=64), AL.add, AL.mult)
        # upper half holds t shifted up one row (for the dy0/dy1 tap pairing)
        nc.sync.dma_start(
            out=t_pad[64:128, (nt * 8) * 66:(nt * 8 + 8) * 66],
            in_=t_pad[0:Cm, (nt * 8 + 1) * 66:(nt * 8 + 9) * 66])

    def conv2(nt):
        r0 = nt * 8
        ps = conv_ps.tile([E, 512], fp32, name="c2ps", tag="conv")
        for dx in range(3):
            rhsp = t_pad[:].rearrange("c (r z) -> c r z", z=66)[:, r0: r0 + 8, dx: dx + 64]
            nc.tensor.matmul(ps[:], ewp[:, dx * E:(dx + 1) * E], rhsp,
                             start=(dx == 0), stop=False)
            rhs2 = t_pad[0:Cm, :].rearrange("c (r z) -> c r z", z=66)[:, r0 + 2: r0 + 10, dx: dx + 64]
            nc.tensor.matmul(ps[:], ew2[:, dx * E:(dx + 1) * E], rhs2,
                             start=False, stop=(dx == 2))
        nc.scalar.activation(out=e_sb[:, nt * 512:(nt + 1) * 512], in_=ps[:],
                             func=AF.Exp, bias=eb[:], scale=1.0)

    def mask_tile(ti):
        p0 = ti * 128
        pt = mt_ps.tile([128, E], fp32r, name="mt", tag="mt")
        nc.tensor.transpose(pt[:], e_sb[:, p0:p0 + 128], id32[0:E, 0:E])
        nc.scalar.copy(out=maskT[:, ti, :], in_=pt[:])
        # maskT free layout: ch = ij*4 + cl
        v_cl_ij = maskT[:, ti, :].rearrange("p (ij cl) -> p cl ij", cl=4)
        s = rsum[:, ti, :]
        nc.vector.tensor_reduce(out=s, in_=v_cl_ij, axis=mybir.AxisListType.X, op=AL.add)
        nc.vector.reciprocal(s, s)
        e_cl_ij = edge[:].unsqueeze(1).broadcast_to([128, 4, K * K])
        nc.vector.tensor_tensor(v_cl_ij, v_cl_ij, e_cl_ij, AL.mult)
        v_ij_cl = maskT[:, ti, :].rearrange("p (ij cl) -> p ij cl", cl=4)
        r_b = rsum[:, ti, :].unsqueeze(1).broadcast_to([128, K * K, 4])
        nc.vector.tensor_tensor(v_ij_cl, v_ij_cl, r_b, AL.mult)

    # ---- main reassembly ----

    win_cache = {}

    def get_window(r, dj):
        # odd padded row r: window rows (r, r+1) live in xT2 block (r-1)//2
        key = (r, dj)
        if key not in win_cache:
            b = (r - 1) // 2
            if dj == 0:
                win_cache[key] = xT2[:, b, :]
            else:
                ps = win_ps.tile([128, C], fp32, tag="winps")
                nc.tensor.matmul(ps[:], shm[:, TAU_IDX[dj], :], xT2[:, b, :],
                                 start=True, stop=True)
                w_sb = work.tile([128, C], fp16, tag="win")
                nc.scalar.copy(out=w_sb[:], in_=ps[:])
                win_cache[key] = w_sb[:]
        return win_cache[key]

    preps = {}

    def tile_prep(ti):
        nonlocal win_cache
        h0 = ti * 2
        # windows for the odd-row DVE taps (i = 1, 3)
        wins = {}
        for i in (1, 3):
            for j in range(K):
                wins[i * K + j] = get_window(h0 + i + 2, j - 2)
        # retire windows no longer needed (keep pool pressure bounded)
        win_cache = {k: v for k, v in win_cache.items() if k[0] >= h0 + 3}

        # column-shifted mask copies for the even-row PE taps:
        # colsB[.., 132*j : 132*j+100] = maskT[:, ti] shifted by -(j-2)
        # partitions (cast to fp16).
        colsB = colsp.tile([128, COLS_FLAT], fp16, name="colsB", tag="colsB")
        nc.scalar.copy(out=colsB[:, 2 * COLS_ROW:2 * COLS_ROW + E],
                       in_=maskT[:, ti, :])
        for j in (0, 1, 3, 4):
            t = 2 - j
            cps = mt_ps.tile([128, E], fp32, name="cps", tag="mt")
            nc.tensor.matmul(cps[:], shm[:, TAU_IDX[t], :],
                             colsB[:, 2 * COLS_ROW:2 * COLS_ROW + E],
                             start=True, stop=True)
            nc.scalar.copy(out=colsB[:, j * COLS_ROW:j * COLS_ROW + E], in_=cps[:])

        # diagonal matrices sws[q, j, a, p] = cols_j[q] if q == p + (j-2):
        # j != 2 via two batched gpsimd affine_selects per cl; j == 2 (no
        # column shift) on ACT as identity-scaled copies (scale = mask col).
        sws_cl = []
        for cl in range(4):
            sws = swsp.tile([128, K, 3, 128], fp16, name=f"sws{cl}", tag="sws")
            for j in (0, 1, 3, 4):
                in_ap = raw_ap(colsB[:], [[40, 3], [0, 128]],
                               extra_offset=cl + j * COLS_JSTRIDE)
                nc.gpsimd.affine_select(
                    out=sws[:, j, :, :], in_=in_ap,
                    compare_op=AL.is_equal, fill=0.0, base=2 - j,
                    channel_multiplier=1, pattern=[[0, 3], [-1, 128]])
            for a in range(3):
                ch = (10 * a + 2) * 4 + cl
                nc.scalar.activation(out=sws[:, 2, a, :], in_=ident16,
                                     func=AF.Copy, bias=0.0,
                                     scale=maskT[:, ti, ch:ch + 1])
            sws_cl.append(sws)
        preps[ti] = (wins, sws_cl)

    def tile_out(ti):
        h0 = ti * 2
        wins, sws_cl = preps.pop(ti)
        stg_tiles = {(di, ch): stagep.tile([96, 2, S * W], fp32, name=f"stg{di}_{ch}", tag=f"stg{di}_{ch}")
                     for di in range(2) for ch in range(2)}
        for di in range(2):
            accs = []
            for dj in range(2):
                cl = di * 2 + dj
                # odd-row taps: DVE fused multiply-accumulate in fp16
                acc = accp.tile([128, C], fp16, tag="acc")
                first = True
                for ij in ODD_TAPS:
                    col = maskT[:, ti, ij * 4 + cl:ij * 4 + cl + 1]
                    if first:
                        nc.vector.tensor_scalar(acc[:], wins[ij], col, None, AL.mult)
                        first = False
                    else:
                        nc.vector.scalar_tensor_tensor(acc[:], wins[ij], col, acc[:],
                                                       AL.mult, AL.add)
                accs.append((dj, cl, acc))
            # transpose accs -> [c, pix] and accumulate the even-row taps on
            # PE, writing the two dj subpixel columns interleaved (stride 2)
            # into one PSUM tile so a single ACT copy stages both.
            for ch in range(2):
                c0 = ch * 96
                ptp = out_ps.tile([96, 2 * 128], fp32, tag="ot")
                for dj, cl, acc in accs:
                    ptv = ptp[:].rearrange("c (p t) -> c p t", t=2)[:, :, dj]
                    sws = sws_cl[cl]
                    nc.tensor.matmul(ptv, acc[:, c0:c0 + 96], ident16,
                                     start=True, stop=False,
                                     skip_group_check=True)
                    for i in (0, 2, 4):
                        q = (h0 + i + 2) // 2
                        for j in range(K):
                            nc.tensor.matmul(ptv, xT[:, q, c0:c0 + 96],
                                             sws[:, j, i // 2, :],
                                             start=False,
                                             stop=(i == 4 and j == K - 1),
                                             skip_group_check=True)
                stg = stg_tiles[(di, ch)]
                nc.scalar.copy(out=stg[:], in_=ptp[:].rearrange("c (h w) -> c h w", h=2))
                nc.sync.dma_start(
                    out=out_d[c0:c0 + 96, h0:h0 + 2, di, :],
                    in_=stg[:])

    # interleave convs and mask tiles; emit the first two tiles' prep
    # (windows + colsB + diagonal builds) early so the Pool/ACT reassembly
    # pipeline starts during the conv phase instead of after it.
    conv1(0)
    for nt in range(8):
        if nt + 1 < 8:
            conv1(nt + 1)
        conv2(nt)
        for sub in range(4):
            ti = nt * 4 + sub
            mask_tile(ti)
            if ti < 2:
                tile_prep(ti)
    for ti in range(NT):
        if ti not in preps:
            tile_prep(ti)
        tile_out(ti)
    es.pop_all().close()


def _host_prep(inputs):
    def fold(w, g, b, m, v):
        s = g / np.sqrt(v + EPS)
        return (w * s[:, None, None, None]).astype(np.float32), (b - m * s).astype(np.float32)

    comp_w_eff, comp_b_eff = fold(inputs["comp_w"], inputs["comp_g"], inputs["comp_b"],
                                  inputs["comp_m"], inputs["comp_v"])
    enc_w_eff, enc_b_eff = fold(inputs["enc_w"], inputs["enc_g"], inputs["enc_b"],
                                inputs["enc_m"], inputs["enc_v"])
    cw = np.ascontiguousarray(comp_w_eff[:, :, 0, 0].T)          # [192, 64]
    cb = comp_b_eff.reshape(Cm, 1)
    ewp = np.concatenate([np.concatenate([enc_w_eff[:, :, 0, dx].T,
                                          enc_w_eff[:, :, 1, dx].T], axis=0)
                          for dx in range(3)], axis=1)  # [128, 300]
    ewp = np.ascontiguousarray(ewp)
    ew2 = np.ascontiguousarray(np.concatenate(
        [enc_w_eff[:, :, 2, dx].T for dx in range(3)], axis=1))  # [64, 300]
    eb = enc_b_eff.reshape(E, 1)
    wv = np.arange(128) % 64
    edge = np.zeros((128, K * K), np.float32)
    for j in range(K):
        ok = (wv + j - 2 >= 0) & (wv + j - 2 < W)
        for i in range(K):
            edge[:, i * K + j] = ok
    shm = np.zeros((128, len(TAUS), 128), np.float16)
    for t, i in TAU_IDX.items():
        shm[:, i, :] = np.eye(128, dtype=np.float16, k=-t)
    shm = shm.reshape(128, len(TAUS) * 128)
    id32 = np.eye(128, dtype=np.float32)
    return dict(cw=cw, cb=cb, ewp=ewp, ew2=ew2, eb=eb, edge=edge, shm=shm, id32=id32)


def kernel(**inputs):
    from concourse.bass_utils import run_bass_kernel_spmd

    inputs = {k: np.asarray(v, dtype=np.float32) for k, v in inputs.items()}
    w = _host_prep(inputs)
    if "nc" not in _prog_cache:
        _prog_cache["nc"] = _build_program()
    nc = _prog_cache["nc"]
    x = inputs["x"]
    in_maps = [dict(x=np.ascontiguousarray(x[b].reshape(C, H * W)), **w) for b in range(B)]
    res = run_bass_kernel_spmd(nc, in_maps, list(range(B)))
    out = np.stack([res.results[b]["out"].reshape(C, 2 * H, 2 * W) for b in range(B)])
    return out

